# revision 59
# baseline (speedup 1.0000x reference)
"""Trainium2 Bass kernel for nn_Attention (general-score attention energies +
softmax over the batch axis).

Math (reference):
    proj     = einsum('lbh,oh->lbo', enc, W) + b      # [L, B, H]
    energies = einsum('bh,lbh->bl', hidden, proj)     # [B, L]
    attn     = softmax(energies, axis=0)[:, None, :]  # [B, 1, L]

Algebraic rewrite:
    energies[b, l] = (hidden @ W)[b] . enc[l, b] + hidden[b] . b
which removes the O(L*B*H*H) projection matmul; the kernel is a memory-bound
stream over enc (fp16 wire format, fp32 accumulation on device).

v2 architecture (h-major stream, PE-reduce):
    enc ships in h-major layout [hp=128][hc, l, b] so the dot-product
    reduction over h becomes a PARTITION-axis sum. The stream is then:
      DVE : one fp16 2x tensor_tensor multiply per supergroup
            (enc_tile *= u_T broadcast over l), the only full-data pass
            on a 0.96 GHz engine;
      PE  : ones-column stationary matmul (lhsT = ones[128,1]) reduces
            each product column at 1 col/cycle @ 2.4 GHz, accumulating
            the 8 h-chunks of each column group in PSUM (start/stop);
      ACT : evicts [1, 512] fp32 energy fragments PSUM -> SBUF.
    Energies land l-major on one partition; a tiny SBUF->SBUF DMA
    redistributes them to [l-part, b-cols], where the batch-axis softmax
    is a free-axis reduce. PE transposes attn back to [b, l] for output.
    Per-core engine busy: DMA ~51us (the HBM-per-NC 358 GB/s wall),
    DVE ~35us, PE ~40us, ACT ~11us -- the stream hides under the DMA.

    u_T = (hidden @ W)^T is computed on PE from 128x128 W blocks as W
    streams in (h-chunk-major), so the stream can start as soon as the
    first supergroup lands; W/setup ship ahead of enc on the same ring.

Distribution: enc is sharded along L across 8 cores (128 l-values per
core). The softmax is over the batch axis (per l), so every core's
softmax is fully local -- no collectives. hidden / W / b are replicated.
"""

import numpy as np

import concourse.bass as bass
import concourse.bacc as bacc
import concourse.tile as tile
from concourse import mybir
from concourse.bass_utils import run_bass_kernel_spmd

F32 = mybir.dt.float32
F16 = mybir.dt.float16

B = 64          # batch
H = 1024        # hidden dim
L = 1024        # enc_len
NCORES = 8
LS = L // NCORES            # 128 l-values per core
KH = 8                      # h-chunks of 128
# supergroup sizes in l-values; first 5 cover l 0:64 (softmax half 0).
# sg0 is small because the DVE stream start is gated by setup+W+sg0 bytes
# through the DMA ramp; 16-l tiles elsewhere for full-line-rate DMA rows
SG_L = [8, 16, 16, 16, 8, 16, 16, 16, 8, 4, 2, 2]
assert sum(SG_L) == LS and sum(SG_L[:5]) == LS // 2
MULT = mybir.AluOpType.mult
ADD = mybir.AluOpType.add

# setup16 column map: [0:512) hidT (8 o-chunks x 64 b), [512:520) bias
# (8 o-chunks), [520:521) ones column, [528:656) all-ones block (row 0
# used as the K=1 lhsT that broadcasts c over 128 partitions)
SET_COLS = 656


def build_program() -> bacc.Bacc:
    nc = bacc.Bacc(
        "TRN2", target_bir_lowering=False, debug=False, num_devices=NCORES
    )

    setup16_p = nc.declare_dram_parameter("setup16", [128, SET_COLS], F16, isOutput=False)
    setup32_p = nc.declare_dram_parameter("setup32", [128, 128], F32, isOutput=False)
    w_p = nc.declare_dram_parameter("w", [128, KH * H], F16, isOutput=False)
    enc_p = nc.declare_dram_parameter("enc", [128, LS * B * KH], F16, isOutput=False)
    out_p = nc.declare_dram_parameter("out", [B, LS], F32, isOutput=True)

    # NOTE: must be built as bacc.Bacc + nc.compile() -- the staged walrus
    # rejects multi-wait instructions emitted by raw Bass+Tile; bacc
    # legalizes them.
    with tile.TileContext(nc) as tc:
        with (
            tc.tile_pool(name="const", bufs=1) as cp,
            tc.tile_pool(name="stream", bufs=8) as sp,
            tc.tile_pool(name="pse", bufs=6, space="PSUM") as pe_pool,
            tc.tile_pool(name="psu", bufs=1, space="PSUM") as ppu,
        ):
            # ---- setup + W DMAs, ahead of enc on the sync ring ----
            setup16 = cp.tile([128, SET_COLS], F16)
            setup32 = cp.tile([128, 128], F32)
            nc.sync.dma_start(setup16[:], setup16_p.ap())
            hidT = setup16[:, 0:512]          # [128, (j, b)] o-chunk j
            biasT = setup16[:, 512:520]       # [128, 8]
            ones_col = setup16[:, 520:521]    # [128, 1]
            ones_row = setup16[0:1, 528:656]  # [1, 128]
            idn = setup32

            wt = cp.tile([128, KH * H], F16)  # [128, (j, k, c)] o-chunk-major
            # W in halves (8 KB/partition rows, near line rate): the j 0-3
            # u matmuls overlap the second half's arrival
            for h2 in range(2):
                nc.sync.dma_start(
                    wt[:, h2 * 4 * H : (h2 + 1) * 4 * H],
                    w_p.ap()[:, h2 * 4 * H : (h2 + 1) * 4 * H],
                )
            # idn is only needed by the u transposes, well after W
            nc.sync.dma_start(setup32[:], setup32_p.ap())

            # ---- u = hidden @ W with hidden stationary (one 64-col
            # LDWEIGHTS per o-chunk, two wide-N matmuls per j); tile range
            # deps gate matmul j on its W half only, so j 0-3 run while W's
            # second half is still in flight. Then 8 PE transposes to
            # u_T[hp, (k, b)] fp16 for the stream multiply.
            psum_uA = pe_pool.tile([B, 512], F32, tag="psum_e")
            psum_uB = pe_pool.tile([B, 512], F32, tag="psum_e")
            u32 = cp.tile([B, H], F32)
            for j in range(KH):
                lhs = hidT[:, 64 * j : 64 * (j + 1)]
                nc.tensor.matmul(
                    psum_uA[:], lhsT=lhs, rhs=wt[:, j * H : j * H + 512],
                    start=(j == 0), stop=(j == KH - 1),
                )
                nc.tensor.matmul(
                    psum_uB[:], lhsT=lhs, rhs=wt[:, j * H + 512 : (j + 1) * H],
                    start=(j == 0), stop=(j == KH - 1),
                )
            nc.scalar.copy(u32[:, 0:512], psum_uA[:])
            # uB eviction on DVE: first DVE instruction, resolves before the
            # first stream multiply's deps, so no head-of-line blocking
            nc.vector.tensor_copy(u32[:, 512:1024], psum_uB[:])
            u16T = cp.tile([128, 512], F16)
            for k in range(KH):
                psum_t = ppu.tile([128, B], F32, tag=f"ut{k % 2}")
                nc.tensor.transpose(
                    psum_t[:], u32[:, 128 * k : 128 * (k + 1)], idn[0:B, 0:B]
                )
                nc.scalar.copy(u16T[:, 64 * k : 64 * (k + 1)], psum_t[:])
            u3 = u16T[:].rearrange("p (k b) -> p k b", k=KH)

            # ---- c[b] = hidden[b] . bias, then broadcast over partitions;
            # emitted lazily after the first supergroups so it stays off the
            # PE/Scalar critical path (softmax only needs it much later) ----
            crep = cp.tile([128, B], F32)

            def emit_c_chain():
                psum_c = ppu.tile([1, B], F32, tag="ut0")
                for j in range(KH):
                    nc.tensor.matmul(
                        psum_c[:],
                        lhsT=biasT[:, j : j + 1],
                        rhs=hidT[:, 64 * j : 64 * (j + 1)],
                        start=(j == 0),
                        stop=(j == KH - 1),
                    )
                c_sb = cp.tile([1, B], F16)
                nc.scalar.copy(c_sb[:], psum_c[:])
                psum_crep = ppu.tile([128, B], F32, tag="ut1")
                nc.tensor.matmul(
                    psum_crep[:], lhsT=ones_row, rhs=c_sb[:], start=True, stop=True
                )
                nc.scalar.copy(crep[:], psum_crep[:])

            # ---- stream state ----
            e_sb = cp.tile([1, LS * B], F32)     # energies, l-major on part 0
            out_sb = cp.tile([B, LS], F32)

            # energies-transposed workspace tiles, one per softmax half;
            # filled by small per-supergroup SBUF->SBUF redistribute DMAs as
            # soon as each supergroup's evictions land, so the softmax only
            # waits on the last tiny one
            eth0 = cp.tile([LS // 2, B], F32)
            eth1 = cp.tile([LS // 2, B], F32)

            def redistribute(l0, ls):
                et_h = eth0 if l0 < LS // 2 else eth1
                lr = l0 % (LS // 2)
                nc.scalar.dma_start(
                    et_h[lr : lr + ls, :],
                    e_sb[0:1, l0 * B : (l0 + ls) * B].rearrange(
                        "p (l b) -> p l b", b=B
                    ),
                )

            def softmax_rows(hf, r0, r1, psum_o):
                # softmax along the free axis for l-rows [64hf+r0, 64hf+r1)
                # of half hf's [l-part, b-cols] workspace, into attn{hf}
                et_h = eth0 if hf == 0 else eth1
                nr = r1 - r0
                nc.vector.tensor_tensor(
                    out=et_h[r0:r1, :], in0=et_h[r0:r1, :], in1=crep[r0:r1, :],
                    op=ADD,
                )
                negm = cp.tile([LS // 2, 1], F32, tag=f"negm{hf}")
                nc.vector.tensor_reduce(
                    out=negm[r0:r1, :],
                    in_=et_h[r0:r1, :],
                    axis=mybir.AxisListType.X,
                    op=mybir.AluOpType.max,
                    negate=True,
                )
                pexp = cp.tile([LS // 2, B], F32, tag=f"pexp{hf}")
                ssum = cp.tile([LS // 2, 1], F32, tag=f"ssum{hf}")
                nc.scalar.activation(
                    pexp[r0:r1, :],
                    et_h[r0:r1, :],
                    mybir.ActivationFunctionType.Exp,
                    bias=negm[r0:r1, 0:1],
                    scale=1.0,
                    accum_out=ssum[r0:r1, :],
                )
                rs = cp.tile([LS // 2, 1], F32, tag=f"rs{hf}")
                nc.vector.reciprocal(rs[r0:r1, :], ssum[r0:r1, :])
                attn = cp.tile([LS // 2, B], F32, tag=f"attn{hf}")
                nc.vector.tensor_scalar_mul(
                    attn[r0:r1, :], pexp[r0:r1, :], rs[r0:r1, 0:1]
                )
                nc.tensor.transpose(
                    psum_o[:, r0:r1], attn[r0:r1, :], idn[r0:r1, r0:r1]
                )

            def softmax_out(hf, psum_o):
                nc.vector.tensor_copy(out_sb[:, 64 * hf : 64 * hf + 64], psum_o[:])
                # half 0 must stay OFF the sync ring: its wait would
                # head-of-line-block the remaining enc DMA issues behind it
                # in the ring FIFO; half 1 is emitted after all enc issues
                eng = nc.scalar if hf == 0 else nc.sync
                eng.dma_start(
                    out_p.ap()[:, 64 * hf : 64 * hf + 64],
                    out_sb[:, 64 * hf : 64 * hf + 64],
                )

            def softmax_half(hf):
                psum_o = ppu.tile([B, LS // 2], F32, tag="ut0")
                softmax_rows(hf, 0, LS // 2, psum_o)
                softmax_out(hf, psum_o)

            # ---- main stream ----
            l0 = 0
            off = 0
            for s, ls in enumerate(SG_L):
                cols = KH * ls * B           # tile cols (k, l, b)
                gcols = ls * B               # cols per h-chunk
                et = sp.tile([128, KH * 16 * B], F16, tag="et")
                nc.sync.dma_start(
                    et[:, 0:cols], enc_p.ap()[:, off : off + cols]
                )
                # per-h-chunk DVE multiply (fp16 2x, in place) interleaved
                # with the PE ones-reduce MMs of that chunk, so the PE gets
                # work every ~0.6us and HAM never re-throttles; the [1, 512]
                # PSUM groups stay open across the 8 chunks (start/stop)
                ngrp = (gcols + 511) // 512
                psum_es = []
                for g in range(ngrp):
                    psum_eg = pe_pool.tile([1, 512], F32, tag="psum_e")
                    psum_es.append(psum_eg)
                # chunk-pair DVE multiply: big enough to amortize the DVE
                # per-op overhead, small enough that the PE gets MM work
                # every ~1.2us (keeps HAM at K=8/8); tiny tail supergroups
                # use a single multiply to minimize per-op overhead
                # sg0: kstep=2 so the first multiply only needs u_T chunks
                # 0-1 (shortens the startup dependency chain)
                kstep = KH if ls <= 4 else (2 if s < 1 else 4)
                veng = nc.vector
                for k in range(0, KH, kstep):
                    ek = et[:, k * gcols : (k + kstep) * gcols].rearrange(
                        "p (ks l b) -> p ks l b", ks=kstep, b=B
                    )
                    veng.tensor_tensor(
                        out=ek,
                        in0=ek,
                        in1=u3[:, k : k + kstep, None, :].broadcast_to(
                            (128, kstep, ls, B)
                        ),
                        op=MULT,
                    )
                    for k2 in range(k, k + kstep):
                        for g in range(ngrp):
                            gn = min(512, gcols - 512 * g)
                            nc.tensor.matmul(
                                psum_es[g][:, 0:gn],
                                lhsT=ones_col,
                                rhs=et[:, k2 * gcols + 512 * g : k2 * gcols + 512 * g + gn],
                                start=(k2 == 0),
                                stop=(k2 == KH - 1),
                            )
                for g in range(ngrp):
                    gn = min(512, gcols - 512 * g)
                    nc.scalar.copy(
                        e_sb[0:1, l0 * B + 512 * g : l0 * B + 512 * g + gn],
                        psum_es[g][:, 0:gn],
                    )
                # tail supergroups share one batched redistribute: one issue
                # + one completion receipt on the exit-critical path
                if s < 9:
                    redistribute(l0, ls)
                elif s == len(SG_L) - 1:
                    redistribute(120, 8)
                l0 += ls
                off += cols
                if s == 1:
                    emit_c_chain()
                # half-0 softmax is emitted a few supergroups after its data
                # is complete: by then the half-0 redistribute receipts have
                # long landed, so its DVE ops can't head-of-line-block the
                # stream multiplies behind them in the DVE FIFO
                if s == 6:
                    softmax_half(0)
                # half-1 rows 0:32 (l 64:96) are complete after sg6's
                # redistribute; doing them here leaves only a [32, 64]
                # softmax slice on the exit-critical path
                if s == 8:
                    psum_o1 = ppu.tile([B, LS // 2], F32, tag="ut0")
                    softmax_rows(1, 0, 32, psum_o1)
            softmax_rows(1, 32, LS // 2, psum_o1)
            softmax_out(1, psum_o1)

    nc.compile()
    return nc


_IDENT = np.eye(128, dtype=np.float32)
_NC_CACHE = []


def _get_nc() -> bacc.Bacc:
    if not _NC_CACHE:
        _NC_CACHE.append(build_program())
    return _NC_CACHE[0]


def make_in_maps(hidden, encoder_outputs, W, b):
    # host side does only layout transforms + fp16 dtype compression; all
    # FLOPs stay on device (fp32 accumulation)
    hidden = np.asarray(hidden, dtype=np.float32)
    # hidT [128, (j, b)]: hidT[p, j*64+b] = hidden[b, 128j+p]
    hidT = hidden.T.reshape(KH, 128, B).transpose(1, 0, 2).reshape(128, 512)
    biasT = np.asarray(b, dtype=np.float32).reshape(KH, 128).T  # [128, 8]
    setup16 = np.zeros((128, SET_COLS), dtype=np.float16)
    setup16[:, 0:512] = hidT.astype(np.float16)
    setup16[:, 512:520] = biasT.astype(np.float16)
    setup16[:, 520:521] = 1.0
    setup16[:, 528:656] = 1.0
    # W o-chunk-major: w16[p, j*1024 + (128k+c)] = W[128j+p, 128k+c]
    w16 = (
        np.asarray(W, dtype=np.float32)
        .astype(np.float16)
        .reshape(KH, 128, H)            # [j, p, h]
        .transpose(1, 0, 2)             # [p, j, h]
        .reshape(128, KH * H)
    )
    w16 = np.ascontiguousarray(w16)
    setup32 = _IDENT

    enc16 = np.asarray(encoder_outputs, dtype=np.float32).astype(np.float16)
    in_maps = []
    for i in range(NCORES):
        # shard [l, b, h] -> [p, k, l, b] -> per-sg slabs [p, (k, l, b)]
        shard = enc16[i * LS : (i + 1) * LS].reshape(LS, B, KH, 128)
        shard = shard.transpose(3, 2, 0, 1)  # [p, k, l, b]
        slabs = []
        l0 = 0
        for ls in SG_L:
            slabs.append(
                np.ascontiguousarray(shard[:, :, l0 : l0 + ls, :]).reshape(128, -1)
            )
            l0 += ls
        encc = np.concatenate(slabs, axis=1)
        in_maps.append(
            {
                "setup16": setup16,
                "setup32": setup32,
                "w": w16,
                "enc": np.ascontiguousarray(encc),
            }
        )
    return in_maps


def kernel(hidden, encoder_outputs, W, b):
    nc = _get_nc()
    in_maps = make_in_maps(hidden, encoder_outputs, W, b)
    res = run_bass_kernel_spmd(nc, in_maps, core_ids=list(range(NCORES)))
    out = np.concatenate([res.results[i]["out"] for i in range(NCORES)], axis=1)
    return out[:, None, :].astype(np.float32)


# revision 60
# speedup vs baseline: 1.0087x; 1.0087x over previous
"""Trainium2 Bass kernel for nn_Attention (general-score attention energies +
softmax over the batch axis).

Math (reference):
    proj     = einsum('lbh,oh->lbo', enc, W) + b      # [L, B, H]
    energies = einsum('bh,lbh->bl', hidden, proj)     # [B, L]
    attn     = softmax(energies, axis=0)[:, None, :]  # [B, 1, L]

Algebraic rewrite:
    energies[b, l] = (hidden @ W)[b] . enc[l, b] + hidden[b] . b
which removes the O(L*B*H*H) projection matmul; the kernel is a memory-bound
stream over enc (fp16 wire format, fp32 accumulation on device).

v2 architecture (h-major stream, PE-reduce):
    enc ships in h-major layout [hp=128][hc, l, b] so the dot-product
    reduction over h becomes a PARTITION-axis sum. The stream is then:
      DVE : one fp16 2x tensor_tensor multiply per supergroup
            (enc_tile *= u_T broadcast over l), the only full-data pass
            on a 0.96 GHz engine;
      PE  : ones-column stationary matmul (lhsT = ones[128,1]) reduces
            each product column at 1 col/cycle @ 2.4 GHz, accumulating
            the 8 h-chunks of each column group in PSUM (start/stop);
      ACT : evicts [1, 512] fp32 energy fragments PSUM -> SBUF.
    Energies land l-major on one partition; a tiny SBUF->SBUF DMA
    redistributes them to [l-part, b-cols], where the batch-axis softmax
    is a free-axis reduce. PE transposes attn back to [b, l] for output.
    Per-core engine busy: DMA ~51us (the HBM-per-NC 358 GB/s wall),
    DVE ~35us, PE ~40us, ACT ~11us -- the stream hides under the DMA.

    u_T = (hidden @ W)^T is computed on PE from 128x128 W blocks as W
    streams in (h-chunk-major), so the stream can start as soon as the
    first supergroup lands; W/setup ship ahead of enc on the same ring.

Distribution: enc is sharded along L across 8 cores (128 l-values per
core). The softmax is over the batch axis (per l), so every core's
softmax is fully local -- no collectives. hidden / W / b are replicated.
"""

import numpy as np

import concourse.bass as bass
import concourse.bacc as bacc
import concourse.tile as tile
from concourse import mybir
from concourse.bass_utils import run_bass_kernel_spmd

F32 = mybir.dt.float32
F16 = mybir.dt.float16

B = 64          # batch
H = 1024        # hidden dim
L = 1024        # enc_len
NCORES = 8
LS = L // NCORES            # 128 l-values per core
KH = 8                      # h-chunks of 128
# supergroup sizes in l-values; first 5 cover l 0:64 (softmax half 0).
# sg0 is small because the DVE stream start is gated by setup+W+sg0 bytes
# through the DMA ramp; 16-l tiles elsewhere for full-line-rate DMA rows
SG_L = [8, 16, 16, 16, 8, 16, 16, 16, 8, 4, 2, 2]
assert sum(SG_L) == LS and sum(SG_L[:5]) == LS // 2
MULT = mybir.AluOpType.mult
ADD = mybir.AluOpType.add

# setup16 column map: [0:512) hidT (8 o-chunks x 64 b), [512:520) bias
# (8 o-chunks), [520:521) ones column, [528:656) all-ones block (row 0
# used as the K=1 lhsT that broadcasts c over 128 partitions)
SET_COLS = 656


def build_program() -> bacc.Bacc:
    nc = bacc.Bacc(
        "TRN2", target_bir_lowering=False, debug=False, num_devices=NCORES
    )

    setup16_p = nc.declare_dram_parameter("setup16", [128, SET_COLS], F16, isOutput=False)
    setup32_p = nc.declare_dram_parameter("setup32", [128, 128], F32, isOutput=False)
    w_p = nc.declare_dram_parameter("w", [128, KH * H], F16, isOutput=False)
    enc_p = nc.declare_dram_parameter("enc", [128, LS * B * KH], F16, isOutput=False)
    out_p = nc.declare_dram_parameter("out", [B, LS], F32, isOutput=True)

    # NOTE: must be built as bacc.Bacc + nc.compile() -- the staged walrus
    # rejects multi-wait instructions emitted by raw Bass+Tile; bacc
    # legalizes them.
    with tile.TileContext(nc) as tc:
        with (
            tc.tile_pool(name="const", bufs=1) as cp,
            tc.tile_pool(name="stream", bufs=8) as sp,
            tc.tile_pool(name="pse", bufs=7, space="PSUM") as pe_pool,
            tc.tile_pool(name="psu", bufs=1, space="PSUM") as ppu,
        ):
            # ---- setup + W DMAs, ahead of enc on the sync ring ----
            setup16 = cp.tile([128, SET_COLS], F16)
            setup32 = cp.tile([128, 128], F32)
            nc.sync.dma_start(setup16[:], setup16_p.ap())
            hidT = setup16[:, 0:512]          # [128, (j, b)] o-chunk j
            biasT = setup16[:, 512:520]       # [128, 8]
            ones_col = setup16[:, 520:521]    # [128, 1]
            ones_row = setup16[0:1, 528:656]  # [1, 128]
            idn = setup32

            wt = cp.tile([128, KH * H], F16)  # [128, (j, k, c)] o-chunk-major
            # W in halves (8 KB/partition rows, near line rate): the j 0-3
            # u matmuls overlap the second half's arrival
            for h2 in range(2):
                nc.sync.dma_start(
                    wt[:, h2 * 4 * H : (h2 + 1) * 4 * H],
                    w_p.ap()[:, h2 * 4 * H : (h2 + 1) * 4 * H],
                )
            # idn is only needed by the u transposes, well after W
            nc.sync.dma_start(setup32[:], setup32_p.ap())

            # ---- u = hidden @ W with hidden stationary (one 64-col
            # LDWEIGHTS per o-chunk, two wide-N matmuls per j); tile range
            # deps gate matmul j on its W half only, so j 0-3 run while W's
            # second half is still in flight. Then 8 PE transposes to
            # u_T[hp, (k, b)] fp16 for the stream multiply.
            psum_uA = pe_pool.tile([B, 512], F32, tag="psum_e")
            psum_uB = pe_pool.tile([B, 512], F32, tag="psum_e")
            u32 = cp.tile([B, H], F32)
            for j in range(KH):
                lhs = hidT[:, 64 * j : 64 * (j + 1)]
                nc.tensor.matmul(
                    psum_uA[:], lhsT=lhs, rhs=wt[:, j * H : j * H + 512],
                    start=(j == 0), stop=(j == KH - 1),
                )
                nc.tensor.matmul(
                    psum_uB[:], lhsT=lhs, rhs=wt[:, j * H + 512 : (j + 1) * H],
                    start=(j == 0), stop=(j == KH - 1),
                )
            nc.scalar.copy(u32[:, 0:512], psum_uA[:])
            # uB eviction on DVE: first DVE instruction, resolves before the
            # first stream multiply's deps, so no head-of-line blocking
            nc.vector.tensor_copy(u32[:, 512:1024], psum_uB[:])
            u16T = cp.tile([128, 512], F16)
            for k in range(KH):
                psum_t = ppu.tile([128, B], F32, tag="ut0")
                nc.tensor.transpose(
                    psum_t[:], u32[:, 128 * k : 128 * (k + 1)], idn[0:B, 0:B]
                )
                nc.scalar.copy(u16T[:, 64 * k : 64 * (k + 1)], psum_t[:])
            u3 = u16T[:].rearrange("p (k b) -> p k b", k=KH)

            # ---- c[b] = hidden[b] . bias, then broadcast over partitions;
            # emitted lazily after the first supergroups so it stays off the
            # PE/Scalar critical path (softmax only needs it much later) ----
            crep = cp.tile([128, B], F32)

            def emit_c_chain():
                psum_c = ppu.tile([1, B], F32, tag="ut0")
                for j in range(KH):
                    nc.tensor.matmul(
                        psum_c[:],
                        lhsT=biasT[:, j : j + 1],
                        rhs=hidT[:, 64 * j : 64 * (j + 1)],
                        start=(j == 0),
                        stop=(j == KH - 1),
                    )
                c_sb = cp.tile([1, B], F16)
                nc.scalar.copy(c_sb[:], psum_c[:])
                psum_crep = ppu.tile([128, B], F32, tag="ut0")
                nc.tensor.matmul(
                    psum_crep[:], lhsT=ones_row, rhs=c_sb[:], start=True, stop=True
                )
                nc.scalar.copy(crep[:], psum_crep[:])

            # ---- stream state ----
            e_sb = cp.tile([1, LS * B], F32)     # energies, l-major on part 0
            out_sb = cp.tile([B, LS], F32)

            # energies-transposed workspace tiles, one per softmax half;
            # filled by small per-supergroup SBUF->SBUF redistribute DMAs as
            # soon as each supergroup's evictions land, so the softmax only
            # waits on the last tiny one
            eth0 = cp.tile([LS // 2, B], F32)
            eth1 = cp.tile([LS // 2, B], F32)

            def redistribute(l0, ls):
                et_h = eth0 if l0 < LS // 2 else eth1
                lr = l0 % (LS // 2)
                nc.scalar.dma_start(
                    et_h[lr : lr + ls, :],
                    e_sb[0:1, l0 * B : (l0 + ls) * B].rearrange(
                        "p (l b) -> p l b", b=B
                    ),
                )

            def softmax_rows(hf, r0, r1, psum_o):
                # softmax along the free axis for l-rows [64hf+r0, 64hf+r1)
                # of half hf's [l-part, b-cols] workspace, into attn{hf}
                et_h = eth0 if hf == 0 else eth1
                nr = r1 - r0
                nc.vector.tensor_tensor(
                    out=et_h[r0:r1, :], in0=et_h[r0:r1, :], in1=crep[r0:r1, :],
                    op=ADD,
                )
                negm = cp.tile([LS // 2, 1], F32, tag=f"negm{hf}")
                nc.vector.tensor_reduce(
                    out=negm[r0:r1, :],
                    in_=et_h[r0:r1, :],
                    axis=mybir.AxisListType.X,
                    op=mybir.AluOpType.max,
                    negate=True,
                )
                pexp = cp.tile([LS // 2, B], F32, tag=f"pexp{hf}")
                ssum = cp.tile([LS // 2, 1], F32, tag=f"ssum{hf}")
                nc.scalar.activation(
                    pexp[r0:r1, :],
                    et_h[r0:r1, :],
                    mybir.ActivationFunctionType.Exp,
                    bias=negm[r0:r1, 0:1],
                    scale=1.0,
                    accum_out=ssum[r0:r1, :],
                )
                rs = cp.tile([LS // 2, 1], F32, tag=f"rs{hf}")
                nc.vector.reciprocal(rs[r0:r1, :], ssum[r0:r1, :])
                attn = cp.tile([LS // 2, B], F32, tag=f"attn{hf}")
                nc.vector.tensor_scalar_mul(
                    attn[r0:r1, :], pexp[r0:r1, :], rs[r0:r1, 0:1]
                )
                nc.tensor.transpose(
                    psum_o[:, r0:r1], attn[r0:r1, :], idn[r0:r1, r0:r1]
                )

            def softmax_out(hf, psum_o):
                nc.vector.tensor_copy(out_sb[:, 64 * hf : 64 * hf + 64], psum_o[:])
                # half 0 must stay OFF the sync ring: its wait would
                # head-of-line-block the remaining enc DMA issues behind it
                # in the ring FIFO; half 1 is emitted after all enc issues
                eng = nc.scalar if hf == 0 else nc.sync
                eng.dma_start(
                    out_p.ap()[:, 64 * hf : 64 * hf + 64],
                    out_sb[:, 64 * hf : 64 * hf + 64],
                )

            def softmax_half(hf):
                psum_o = ppu.tile([B, LS // 2], F32, tag="ut0")
                softmax_rows(hf, 0, LS // 2, psum_o)
                softmax_out(hf, psum_o)

            # ---- main stream ----
            l0 = 0
            off = 0
            for s, ls in enumerate(SG_L):
                cols = KH * ls * B           # tile cols (k, l, b)
                gcols = ls * B               # cols per h-chunk
                et = sp.tile([128, KH * 16 * B], F16, tag="et")
                nc.sync.dma_start(
                    et[:, 0:cols], enc_p.ap()[:, off : off + cols]
                )
                # per-h-chunk DVE multiply (fp16 2x, in place) interleaved
                # with the PE ones-reduce MMs of that chunk, so the PE gets
                # work every ~0.6us and HAM never re-throttles; the [1, 512]
                # PSUM groups stay open across the 8 chunks (start/stop)
                ngrp = (gcols + 511) // 512
                psum_es = []
                for g in range(ngrp):
                    psum_eg = pe_pool.tile([1, 512], F32, tag="psum_e")
                    psum_es.append(psum_eg)
                # chunk-pair DVE multiply: big enough to amortize the DVE
                # per-op overhead, small enough that the PE gets MM work
                # every ~1.2us (keeps HAM at K=8/8); tiny tail supergroups
                # use a single multiply to minimize per-op overhead
                # sg0: kstep=2 so the first multiply only needs u_T chunks
                # 0-1 (shortens the startup dependency chain)
                kstep = KH if ls <= 4 else (2 if s < 1 else 4)
                veng = nc.vector
                for k in range(0, KH, kstep):
                    ek = et[:, k * gcols : (k + kstep) * gcols].rearrange(
                        "p (ks l b) -> p ks l b", ks=kstep, b=B
                    )
                    veng.tensor_tensor(
                        out=ek,
                        in0=ek,
                        in1=u3[:, k : k + kstep, None, :].broadcast_to(
                            (128, kstep, ls, B)
                        ),
                        op=MULT,
                    )
                    for k2 in range(k, k + kstep):
                        for g in range(ngrp):
                            gn = min(512, gcols - 512 * g)
                            nc.tensor.matmul(
                                psum_es[g][:, 0:gn],
                                lhsT=ones_col,
                                rhs=et[:, k2 * gcols + 512 * g : k2 * gcols + 512 * g + gn],
                                start=(k2 == 0),
                                stop=(k2 == KH - 1),
                            )
                for g in range(ngrp):
                    gn = min(512, gcols - 512 * g)
                    nc.scalar.copy(
                        e_sb[0:1, l0 * B + 512 * g : l0 * B + 512 * g + gn],
                        psum_es[g][:, 0:gn],
                    )
                # tail supergroups share one batched redistribute: one issue
                # + one completion receipt on the exit-critical path
                if s < 9:
                    redistribute(l0, ls)
                elif s == len(SG_L) - 1:
                    redistribute(120, 8)
                l0 += ls
                off += cols
                if s == 1:
                    emit_c_chain()
                # half-0 softmax is emitted a few supergroups after its data
                # is complete: by then the half-0 redistribute receipts have
                # long landed, so its DVE ops can't head-of-line-block the
                # stream multiplies behind them in the DVE FIFO
                if s == 6:
                    softmax_half(0)
                # half-1 rows 0:32 (l 64:96) are complete after sg6's
                # redistribute; doing them here leaves only a [32, 64]
                # softmax slice on the exit-critical path
                if s == 8:
                    psum_o1 = ppu.tile([B, LS // 2], F32, tag="ut0")
                    softmax_rows(1, 0, 32, psum_o1)
            softmax_rows(1, 32, LS // 2, psum_o1)
            softmax_out(1, psum_o1)

    nc.compile()
    return nc


_IDENT = np.eye(128, dtype=np.float32)
_NC_CACHE = []


def _get_nc() -> bacc.Bacc:
    if not _NC_CACHE:
        _NC_CACHE.append(build_program())
    return _NC_CACHE[0]


def make_in_maps(hidden, encoder_outputs, W, b):
    # host side does only layout transforms + fp16 dtype compression; all
    # FLOPs stay on device (fp32 accumulation)
    hidden = np.asarray(hidden, dtype=np.float32)
    # hidT [128, (j, b)]: hidT[p, j*64+b] = hidden[b, 128j+p]
    hidT = hidden.T.reshape(KH, 128, B).transpose(1, 0, 2).reshape(128, 512)
    biasT = np.asarray(b, dtype=np.float32).reshape(KH, 128).T  # [128, 8]
    setup16 = np.zeros((128, SET_COLS), dtype=np.float16)
    setup16[:, 0:512] = hidT.astype(np.float16)
    setup16[:, 512:520] = biasT.astype(np.float16)
    setup16[:, 520:521] = 1.0
    setup16[:, 528:656] = 1.0
    # W o-chunk-major: w16[p, j*1024 + (128k+c)] = W[128j+p, 128k+c]
    w16 = (
        np.asarray(W, dtype=np.float32)
        .astype(np.float16)
        .reshape(KH, 128, H)            # [j, p, h]
        .transpose(1, 0, 2)             # [p, j, h]
        .reshape(128, KH * H)
    )
    w16 = np.ascontiguousarray(w16)
    setup32 = _IDENT

    enc16 = np.asarray(encoder_outputs, dtype=np.float32).astype(np.float16)
    in_maps = []
    for i in range(NCORES):
        # shard [l, b, h] -> [p, k, l, b] -> per-sg slabs [p, (k, l, b)]
        shard = enc16[i * LS : (i + 1) * LS].reshape(LS, B, KH, 128)
        shard = shard.transpose(3, 2, 0, 1)  # [p, k, l, b]
        slabs = []
        l0 = 0
        for ls in SG_L:
            slabs.append(
                np.ascontiguousarray(shard[:, :, l0 : l0 + ls, :]).reshape(128, -1)
            )
            l0 += ls
        encc = np.concatenate(slabs, axis=1)
        in_maps.append(
            {
                "setup16": setup16,
                "setup32": setup32,
                "w": w16,
                "enc": np.ascontiguousarray(encc),
            }
        )
    return in_maps


def kernel(hidden, encoder_outputs, W, b):
    nc = _get_nc()
    in_maps = make_in_maps(hidden, encoder_outputs, W, b)
    res = run_bass_kernel_spmd(nc, in_maps, core_ids=list(range(NCORES)))
    out = np.concatenate([res.results[i]["out"] for i in range(NCORES)], axis=1)
    return out[:, None, :].astype(np.float32)


# revision 61
# speedup vs baseline: 1.0301x; 1.0212x over previous
"""Trainium2 Bass kernel for nn_Attention (general-score attention energies +
softmax over the batch axis).

Math (reference):
    proj     = einsum('lbh,oh->lbo', enc, W) + b      # [L, B, H]
    energies = einsum('bh,lbh->bl', hidden, proj)     # [B, L]
    attn     = softmax(energies, axis=0)[:, None, :]  # [B, 1, L]

Algebraic rewrite:
    energies[b, l] = (hidden @ W)[b] . enc[l, b] + hidden[b] . b
which removes the O(L*B*H*H) projection matmul; the kernel is a memory-bound
stream over enc (fp16 wire format, fp32 accumulation on device).

v2 architecture (h-major stream, PE-reduce). enc ships in h-major
supergroup slabs [hp=128][(hc, l, b)] so the dot-product reduction over
h becomes a PARTITION-axis sum:
    DVE : fp16 2x in-place tensor_tensor multiplies (enc *= u_T
          broadcast over l), chunk-paired (kstep=4) so the PE gets MM
          work every ~2.2us and HAM stays at K=8/8. This is the only
          full-data compute pass (~40us busy) and the critical engine.
    PE  : ones-column stationary matmul (lhsT = ones[128,1]) reduces
          each product column at 1 col/cycle @ 2.4 GHz warm, the 8
          h-chunks of each [1, 512] column group accumulating in PSUM
          (start/stop); 7 rotating PSUM banks absorb eviction lag.
    ACT : evicts [1, 512] fp32 energy fragments PSUM -> SBUF and issues
          the redistribute DMAs.
Energies land l-major on partition 0; small SBUF->SBUF DMAs scatter
them to [l-part, b-cols] workspaces where the batch-axis softmax is a
free-axis reduce; PE transposes attn back to [b, l] for output.

Schedule notes (hard-won):
  - Engine queues are strict FIFO: any op emitted on DVE/Sync ahead of
    the stream whose deps resolve late head-of-line-blocks everything
    (u-chain casts -> ScalarE; half-0 out-DMA off the sync ring).
  - DMA: 16 KB/partition rows run at ~425-435 GB/s (fabric rate);
    small rows (<=4KB) at ~60-80%; the first ~4 MB pay a ramp. The
    stream start is gated by setup+W+sg0 bytes, so sg0 is 8 l-values
    and W goes in halves with the u matmuls j-gated per half.
  - u = hidden @ W runs with hidden stationary (one 64-col LDWEIGHTS
    per o-chunk, wide-N moving W), then 8 PE transposes produce u_T;
    the whole chain hides in sg0's DMA shadow.
  - Tail: supergroup sizes taper (8,4,2,2), the last 8 l-values share
    one batched redistribute, and softmax half-1 is row-split so only
    a [32, 64] slice remains on the exit-critical path.
  - Run-to-run variance ±3-6us: HAM K=4/8 windows and the firmware
    50%-util activity throttle (P0 downclock ~20% when hot).

Timing (HW, neuron-profile, core 0): 73.6-79.4us over repeated runs
(median ~75us; prior session's baseline: 82.8-85.1us). Engine busy per
core: DMA ~49us active (18.4 MB wire), DVE ~41, PE ~46 (incl. waits),
ACT ~27. fp16 end-to-end rel err ~1.8e-3 (gate 2e-2).

Distribution: enc is sharded along L across 8 cores (128 l-values per
core). The softmax is over the batch axis (per l), so every core's
softmax is fully local -- no collectives. hidden / W / b are replicated.
"""

import numpy as np

import concourse.bass as bass
import concourse.bacc as bacc
import concourse.tile as tile
from concourse import mybir
from concourse.bass_utils import run_bass_kernel_spmd

F32 = mybir.dt.float32
F16 = mybir.dt.float16

B = 64          # batch
H = 1024        # hidden dim
L = 1024        # enc_len
NCORES = 8
LS = L // NCORES            # 128 l-values per core
KH = 8                      # h-chunks of 128
# supergroup sizes in l-values; first 5 cover l 0:64 (softmax half 0).
# sg0 is small because the DVE stream start is gated by setup+W+sg0 bytes
# through the DMA ramp; 16-l tiles elsewhere for full-line-rate DMA rows
SG_L = [8, 16, 16, 16, 8, 16, 16, 16, 8, 4, 2, 2]
assert sum(SG_L) == LS and sum(SG_L[:5]) == LS // 2
MULT = mybir.AluOpType.mult
ADD = mybir.AluOpType.add

# setup16 column map: [0:512) hidT (8 o-chunks x 64 b), [512:520) bias
# (8 o-chunks), [520:521) ones column, [528:656) all-ones block (row 0
# used as the K=1 lhsT that broadcasts c over 128 partitions)
SET_COLS = 656


def build_program() -> bacc.Bacc:
    nc = bacc.Bacc(
        "TRN2", target_bir_lowering=False, debug=False, num_devices=NCORES
    )

    setup16_p = nc.declare_dram_parameter("setup16", [128, SET_COLS], F16, isOutput=False)
    setup32_p = nc.declare_dram_parameter("setup32", [128, 128], F32, isOutput=False)
    w_p = nc.declare_dram_parameter("w", [128, KH * H], F16, isOutput=False)
    enc_p = nc.declare_dram_parameter("enc", [128, LS * B * KH], F16, isOutput=False)
    out_p = nc.declare_dram_parameter("out", [B, LS], F32, isOutput=True)

    # NOTE: must be built as bacc.Bacc + nc.compile() -- the staged walrus
    # rejects multi-wait instructions emitted by raw Bass+Tile; bacc
    # legalizes them.
    with tile.TileContext(nc) as tc:
        with (
            tc.tile_pool(name="const", bufs=1) as cp,
            tc.tile_pool(name="stream", bufs=8) as sp,
            tc.tile_pool(name="pse", bufs=7, space="PSUM") as pe_pool,
            tc.tile_pool(name="psu", bufs=1, space="PSUM") as ppu,
        ):
            # ---- setup + W DMAs, ahead of enc on the sync ring ----
            setup16 = cp.tile([128, SET_COLS], F16)
            setup32 = cp.tile([128, 128], F32)
            nc.sync.dma_start(setup16[:], setup16_p.ap())
            hidT = setup16[:, 0:512]          # [128, (j, b)] o-chunk j
            biasT = setup16[:, 512:520]       # [128, 8]
            ones_col = setup16[:, 520:521]    # [128, 1]
            ones_row = setup16[0:1, 528:656]  # [1, 128]
            idn = setup32

            wt = cp.tile([128, KH * H], F16)  # [128, (j, k, c)] o-chunk-major
            # W in halves (8 KB/partition rows, near line rate): the j 0-3
            # u matmuls overlap the second half's arrival
            for h2 in range(2):
                nc.sync.dma_start(
                    wt[:, h2 * 4 * H : (h2 + 1) * 4 * H],
                    w_p.ap()[:, h2 * 4 * H : (h2 + 1) * 4 * H],
                )
            # idn is only needed by the u transposes, well after W
            nc.sync.dma_start(setup32[:], setup32_p.ap())

            # ---- u = hidden @ W with hidden stationary (one 64-col
            # LDWEIGHTS per o-chunk, two wide-N matmuls per j); tile range
            # deps gate matmul j on its W half only, so j 0-3 run while W's
            # second half is still in flight. Then 8 PE transposes to
            # u_T[hp, (k, b)] fp16 for the stream multiply.
            psum_uA = pe_pool.tile([B, 512], F32, tag="psum_e")
            psum_uB = pe_pool.tile([B, 512], F32, tag="psum_e")
            u32 = cp.tile([B, H], F32)
            for j in range(KH):
                lhs = hidT[:, 64 * j : 64 * (j + 1)]
                nc.tensor.matmul(
                    psum_uA[:], lhsT=lhs, rhs=wt[:, j * H : j * H + 512],
                    start=(j == 0), stop=(j == KH - 1),
                )
                nc.tensor.matmul(
                    psum_uB[:], lhsT=lhs, rhs=wt[:, j * H + 512 : (j + 1) * H],
                    start=(j == 0), stop=(j == KH - 1),
                )
            nc.scalar.copy(u32[:, 0:512], psum_uA[:])
            # uB eviction on DVE: first DVE instruction, resolves before the
            # first stream multiply's deps, so no head-of-line blocking
            nc.vector.tensor_copy(u32[:, 512:1024], psum_uB[:])
            u16T = cp.tile([128, 512], F16)
            for k in range(KH):
                psum_t = ppu.tile([128, B], F32, tag="ut0")
                nc.tensor.transpose(
                    psum_t[:], u32[:, 128 * k : 128 * (k + 1)], idn[0:B, 0:B]
                )
                nc.scalar.copy(u16T[:, 64 * k : 64 * (k + 1)], psum_t[:])
            u3 = u16T[:].rearrange("p (k b) -> p k b", k=KH)

            # ---- c[b] = hidden[b] . bias, then broadcast over partitions;
            # emitted lazily after the first supergroups so it stays off the
            # PE/Scalar critical path (softmax only needs it much later) ----
            crep = cp.tile([128, B], F32)

            def emit_c_chain():
                psum_c = ppu.tile([1, B], F32, tag="ut0")
                for j in range(KH):
                    nc.tensor.matmul(
                        psum_c[:],
                        lhsT=biasT[:, j : j + 1],
                        rhs=hidT[:, 64 * j : 64 * (j + 1)],
                        start=(j == 0),
                        stop=(j == KH - 1),
                    )
                c_sb = cp.tile([1, B], F16)
                nc.scalar.copy(c_sb[:], psum_c[:])
                psum_crep = ppu.tile([128, B], F32, tag="ut0")
                nc.tensor.matmul(
                    psum_crep[:], lhsT=ones_row, rhs=c_sb[:], start=True, stop=True
                )
                nc.scalar.copy(crep[:], psum_crep[:])

            # ---- stream state ----
            e_sb = cp.tile([1, LS * B], F32)     # energies, l-major on part 0
            out_sb = cp.tile([B, LS], F32)

            # energies-transposed workspace tiles, one per softmax half;
            # filled by small per-supergroup SBUF->SBUF redistribute DMAs as
            # soon as each supergroup's evictions land, so the softmax only
            # waits on the last tiny one
            eth0 = cp.tile([LS // 2, B], F32)
            eth1 = cp.tile([LS // 2, B], F32)

            def redistribute(l0, ls):
                et_h = eth0 if l0 < LS // 2 else eth1
                lr = l0 % (LS // 2)
                nc.scalar.dma_start(
                    et_h[lr : lr + ls, :],
                    e_sb[0:1, l0 * B : (l0 + ls) * B].rearrange(
                        "p (l b) -> p l b", b=B
                    ),
                )

            def softmax_rows(hf, r0, r1, psum_o):
                # softmax along the free axis for l-rows [64hf+r0, 64hf+r1)
                # of half hf's [l-part, b-cols] workspace, into attn{hf}
                et_h = eth0 if hf == 0 else eth1
                nr = r1 - r0
                nc.vector.tensor_tensor(
                    out=et_h[r0:r1, :], in0=et_h[r0:r1, :], in1=crep[r0:r1, :],
                    op=ADD,
                )
                negm = cp.tile([LS // 2, 1], F32, tag=f"negm{hf}")
                nc.vector.tensor_reduce(
                    out=negm[r0:r1, :],
                    in_=et_h[r0:r1, :],
                    axis=mybir.AxisListType.X,
                    op=mybir.AluOpType.max,
                    negate=True,
                )
                pexp = cp.tile([LS // 2, B], F32, tag=f"pexp{hf}")
                ssum = cp.tile([LS // 2, 1], F32, tag=f"ssum{hf}")
                nc.scalar.activation(
                    pexp[r0:r1, :],
                    et_h[r0:r1, :],
                    mybir.ActivationFunctionType.Exp,
                    bias=negm[r0:r1, 0:1],
                    scale=1.0,
                    accum_out=ssum[r0:r1, :],
                )
                rs = cp.tile([LS // 2, 1], F32, tag=f"rs{hf}")
                nc.vector.reciprocal(rs[r0:r1, :], ssum[r0:r1, :])
                attn = cp.tile([LS // 2, B], F32, tag=f"attn{hf}")
                nc.vector.tensor_scalar_mul(
                    attn[r0:r1, :], pexp[r0:r1, :], rs[r0:r1, 0:1]
                )
                nc.tensor.transpose(
                    psum_o[:, r0:r1], attn[r0:r1, :], idn[r0:r1, r0:r1]
                )

            def softmax_out(hf, psum_o):
                nc.vector.tensor_copy(out_sb[:, 64 * hf : 64 * hf + 64], psum_o[:])
                # half 0 must stay OFF the sync ring: its wait would
                # head-of-line-block the remaining enc DMA issues behind it
                # in the ring FIFO; half 1 is emitted after all enc issues
                eng = nc.scalar if hf == 0 else nc.sync
                eng.dma_start(
                    out_p.ap()[:, 64 * hf : 64 * hf + 64],
                    out_sb[:, 64 * hf : 64 * hf + 64],
                )

            def softmax_half(hf):
                psum_o = ppu.tile([B, LS // 2], F32, tag="ut0")
                softmax_rows(hf, 0, LS // 2, psum_o)
                softmax_out(hf, psum_o)

            # ---- main stream ----
            l0 = 0
            off = 0
            for s, ls in enumerate(SG_L):
                cols = KH * ls * B           # tile cols (k, l, b)
                gcols = ls * B               # cols per h-chunk
                et = sp.tile([128, KH * 16 * B], F16, tag="et")
                nc.sync.dma_start(
                    et[:, 0:cols], enc_p.ap()[:, off : off + cols]
                )
                # per-h-chunk DVE multiply (fp16 2x, in place) interleaved
                # with the PE ones-reduce MMs of that chunk, so the PE gets
                # work every ~0.6us and HAM never re-throttles; the [1, 512]
                # PSUM groups stay open across the 8 chunks (start/stop)
                ngrp = (gcols + 511) // 512
                psum_es = []
                for g in range(ngrp):
                    psum_eg = pe_pool.tile([1, 512], F32, tag="psum_e")
                    psum_es.append(psum_eg)
                # chunk-pair DVE multiply: big enough to amortize the DVE
                # per-op overhead, small enough that the PE gets MM work
                # every ~1.2us (keeps HAM at K=8/8); tiny tail supergroups
                # use a single multiply to minimize per-op overhead
                # sg0: kstep=2 so the first multiply only needs u_T chunks
                # 0-1 (shortens the startup dependency chain)
                kstep = KH if ls <= 4 else (2 if s < 1 else 4)
                veng = nc.vector
                for k in range(0, KH, kstep):
                    ek = et[:, k * gcols : (k + kstep) * gcols].rearrange(
                        "p (ks l b) -> p ks l b", ks=kstep, b=B
                    )
                    veng.tensor_tensor(
                        out=ek,
                        in0=ek,
                        in1=u3[:, k : k + kstep, None, :].broadcast_to(
                            (128, kstep, ls, B)
                        ),
                        op=MULT,
                    )
                    for k2 in range(k, k + kstep):
                        for g in range(ngrp):
                            gn = min(512, gcols - 512 * g)
                            nc.tensor.matmul(
                                psum_es[g][:, 0:gn],
                                lhsT=ones_col,
                                rhs=et[:, k2 * gcols + 512 * g : k2 * gcols + 512 * g + gn],
                                start=(k2 == 0),
                                stop=(k2 == KH - 1),
                            )
                for g in range(ngrp):
                    gn = min(512, gcols - 512 * g)
                    nc.scalar.copy(
                        e_sb[0:1, l0 * B + 512 * g : l0 * B + 512 * g + gn],
                        psum_es[g][:, 0:gn],
                    )
                # tail supergroups share one batched redistribute: one issue
                # + one completion receipt on the exit-critical path
                if s < 9:
                    redistribute(l0, ls)
                elif s == len(SG_L) - 1:
                    redistribute(120, 8)
                l0 += ls
                off += cols
                if s == 1:
                    emit_c_chain()
                # half-0 softmax is emitted a few supergroups after its data
                # is complete: by then the half-0 redistribute receipts have
                # long landed, so its DVE ops can't head-of-line-block the
                # stream multiplies behind them in the DVE FIFO
                if s == 6:
                    softmax_half(0)
                # half-1 rows 0:32 (l 64:96) are complete after sg6's
                # redistribute; doing them here leaves only a [32, 64]
                # softmax slice on the exit-critical path
                if s == 8:
                    psum_o1 = ppu.tile([B, LS // 2], F32, tag="ut0")
                    softmax_rows(1, 0, 32, psum_o1)
            softmax_rows(1, 32, LS // 2, psum_o1)
            softmax_out(1, psum_o1)

    nc.compile()
    return nc


_IDENT = np.eye(128, dtype=np.float32)
_NC_CACHE = []


def _get_nc() -> bacc.Bacc:
    if not _NC_CACHE:
        _NC_CACHE.append(build_program())
    return _NC_CACHE[0]


def make_in_maps(hidden, encoder_outputs, W, b):
    # host side does only layout transforms + fp16 dtype compression; all
    # FLOPs stay on device (fp32 accumulation)
    hidden = np.asarray(hidden, dtype=np.float32)
    # hidT [128, (j, b)]: hidT[p, j*64+b] = hidden[b, 128j+p]
    hidT = hidden.T.reshape(KH, 128, B).transpose(1, 0, 2).reshape(128, 512)
    biasT = np.asarray(b, dtype=np.float32).reshape(KH, 128).T  # [128, 8]
    setup16 = np.zeros((128, SET_COLS), dtype=np.float16)
    setup16[:, 0:512] = hidT.astype(np.float16)
    setup16[:, 512:520] = biasT.astype(np.float16)
    setup16[:, 520:521] = 1.0
    setup16[:, 528:656] = 1.0
    # W o-chunk-major: w16[p, j*1024 + (128k+c)] = W[128j+p, 128k+c]
    w16 = (
        np.asarray(W, dtype=np.float32)
        .astype(np.float16)
        .reshape(KH, 128, H)            # [j, p, h]
        .transpose(1, 0, 2)             # [p, j, h]
        .reshape(128, KH * H)
    )
    w16 = np.ascontiguousarray(w16)
    setup32 = _IDENT

    enc16 = np.asarray(encoder_outputs, dtype=np.float32).astype(np.float16)
    in_maps = []
    for i in range(NCORES):
        # shard [l, b, h] -> [p, k, l, b] -> per-sg slabs [p, (k, l, b)]
        shard = enc16[i * LS : (i + 1) * LS].reshape(LS, B, KH, 128)
        shard = shard.transpose(3, 2, 0, 1)  # [p, k, l, b]
        slabs = []
        l0 = 0
        for ls in SG_L:
            slabs.append(
                np.ascontiguousarray(shard[:, :, l0 : l0 + ls, :]).reshape(128, -1)
            )
            l0 += ls
        encc = np.concatenate(slabs, axis=1)
        in_maps.append(
            {
                "setup16": setup16,
                "setup32": setup32,
                "w": w16,
                "enc": np.ascontiguousarray(encc),
            }
        )
    return in_maps


def kernel(hidden, encoder_outputs, W, b):
    nc = _get_nc()
    in_maps = make_in_maps(hidden, encoder_outputs, W, b)
    res = run_bass_kernel_spmd(nc, in_maps, core_ids=list(range(NCORES)))
    out = np.concatenate([res.results[i]["out"] for i in range(NCORES)], axis=1)
    return out[:, None, :].astype(np.float32)


# revision 62
# speedup vs baseline: 1.0590x; 1.0281x over previous
"""Trainium2 Bass kernel for nn_Attention (general-score attention energies +
softmax over the batch axis).

Math (reference):
    proj     = einsum('lbh,oh->lbo', enc, W) + b      # [L, B, H]
    energies = einsum('bh,lbh->bl', hidden, proj)     # [B, L]
    attn     = softmax(energies, axis=0)[:, None, :]  # [B, 1, L]

Algebraic rewrite:
    energies[b, l] = (hidden @ W)[b] . enc[l, b] + hidden[b] . b
which removes the O(L*B*H*H) projection matmul; the kernel is a memory-bound
stream over enc (fp16 wire format, fp32 accumulation on device).

v2 architecture (h-major stream, PE-reduce). enc ships in h-major
supergroup slabs [hp=128][(hc, l, b)] so the dot-product reduction over
h becomes a PARTITION-axis sum:
    DVE : fp16 2x in-place tensor_tensor multiplies (enc *= u_T
          broadcast over l), chunk-paired (kstep=4) so the PE gets MM
          work every ~2.2us and HAM stays at K=8/8. This is the only
          full-data compute pass (~40us busy) and the critical engine.
    PE  : ones-column stationary matmul (lhsT = ones[128,1]) reduces
          each product column at 1 col/cycle @ 2.4 GHz warm, the 8
          h-chunks of each [1, 512] column group accumulating in PSUM
          (start/stop); 7 rotating PSUM banks absorb eviction lag.
    ACT : evicts [1, 512] fp32 energy fragments PSUM -> SBUF and issues
          the redistribute DMAs.
Energies land l-major on partition 0; small SBUF->SBUF DMAs scatter
them to [l-part, b-cols] workspaces where the batch-axis softmax is a
free-axis reduce; PE transposes attn back to [b, l] for output.

Schedule notes (hard-won):
  - Engine queues are strict FIFO: any op emitted on DVE/Sync ahead of
    the stream whose deps resolve late head-of-line-blocks everything
    (u-chain casts -> ScalarE; half-0 out-DMA off the sync ring).
  - DMA: 16 KB/partition rows run at ~425-435 GB/s (fabric rate);
    small rows (<=4KB) at ~60-80%; the first ~4 MB pay a ramp. The
    stream start is gated by setup+W+sg0 bytes, so sg0 is 8 l-values
    and W goes in halves with the u matmuls j-gated per half.
  - u = hidden @ W runs with hidden stationary (one 64-col LDWEIGHTS
    per o-chunk, wide-N moving W), then 8 PE transposes produce u_T;
    the whole chain hides in sg0's DMA shadow.
  - Tail: supergroup sizes taper (8,4,2,2), the last 8 l-values share
    one batched redistribute, and softmax half-1 is row-split so only
    a [32, 64] slice remains on the exit-critical path.
  - Run-to-run variance ±3-6us: HAM K=4/8 windows and the firmware
    50%-util activity throttle (P0 downclock ~20% when hot).

Timing (HW, neuron-profile, core 0): 73.6-79.4us over repeated runs
(median ~75us; prior session's baseline: 82.8-85.1us). Engine busy per
core: DMA ~49us active (18.4 MB wire), DVE ~41, PE ~46 (incl. waits),
ACT ~27. fp16 end-to-end rel err ~1.8e-3 (gate 2e-2).

Distribution: enc is sharded along L across 8 cores (128 l-values per
core). The softmax is over the batch axis (per l), so every core's
softmax is fully local -- no collectives. hidden / W / b are replicated.
"""

import numpy as np

import concourse.bass as bass
import concourse.bacc as bacc
import concourse.tile as tile
from concourse import mybir
from concourse.bass_utils import run_bass_kernel_spmd

F32 = mybir.dt.float32
F16 = mybir.dt.float16

B = 64          # batch
H = 1024        # hidden dim
L = 1024        # enc_len
NCORES = 8
LS = L // NCORES            # 128 l-values per core
KH = 8                      # h-chunks of 128
# supergroup sizes in l-values; first 5 cover l 0:64 (softmax half 0).
# sg0 is small because the DVE stream start is gated by setup+W+sg0 bytes
# through the DMA ramp; 16-l tiles elsewhere for full-line-rate DMA rows
SG_L = [8, 16, 16, 16, 8, 16, 16, 16, 8, 4, 2, 2]
assert sum(SG_L) == LS and sum(SG_L[:5]) == LS // 2
MULT = mybir.AluOpType.mult
ADD = mybir.AluOpType.add

# setup16 column map: [0:512) hidT (8 o-chunks x 64 b), [512:520) bias
# (8 o-chunks), [520:521) ones column, [528:656) all-ones block (row 0
# used as the K=1 lhsT that broadcasts c over 128 partitions)
SET_COLS = 656


def build_program() -> bacc.Bacc:
    nc = bacc.Bacc(
        "TRN2", target_bir_lowering=False, debug=False, num_devices=NCORES
    )

    setup16_p = nc.declare_dram_parameter("setup16", [128, SET_COLS], F16, isOutput=False)
    setup32_p = nc.declare_dram_parameter("setup32", [128, 128], F32, isOutput=False)
    w_p = nc.declare_dram_parameter("w", [128, KH * H], F16, isOutput=False)
    enc_p = nc.declare_dram_parameter("enc", [128, LS * B * KH], F16, isOutput=False)
    out_p = nc.declare_dram_parameter("out", [B, LS], F32, isOutput=True)

    # NOTE: must be built as bacc.Bacc + nc.compile() -- the staged walrus
    # rejects multi-wait instructions emitted by raw Bass+Tile; bacc
    # legalizes them.
    with tile.TileContext(nc) as tc:
        with (
            tc.tile_pool(name="const", bufs=1) as cp,
            tc.tile_pool(name="stream", bufs=8) as sp,
            tc.tile_pool(name="pse", bufs=6, space="PSUM") as pe_pool,
            tc.tile_pool(name="psu", bufs=1, space="PSUM") as ppu,
        ):
            # ---- setup + W DMAs, ahead of enc on the sync ring ----
            setup16 = cp.tile([128, SET_COLS], F16)
            setup32 = cp.tile([128, 128], F32)
            nc.sync.dma_start(setup16[:], setup16_p.ap())
            hidT = setup16[:, 0:512]          # [128, (j, b)] o-chunk j
            biasT = setup16[:, 512:520]       # [128, 8]
            ones_col = setup16[:, 520:521]    # [128, 1]
            ones_row = setup16[0:1, 528:656]  # [1, 128]
            idn = setup32

            wt = cp.tile([128, KH * H], F16)  # [128, (j, k, c)] o-chunk-major
            # W in halves (8 KB/partition rows, near line rate): the j 0-3
            # u matmuls overlap the second half's arrival
            for h2 in range(2):
                nc.sync.dma_start(
                    wt[:, h2 * 4 * H : (h2 + 1) * 4 * H],
                    w_p.ap()[:, h2 * 4 * H : (h2 + 1) * 4 * H],
                )
            # idn is only needed by the u transposes, well after W
            nc.sync.dma_start(setup32[:], setup32_p.ap())

            # ---- u = hidden @ W with hidden stationary (one 64-col
            # LDWEIGHTS per o-chunk, two wide-N matmuls per j); tile range
            # deps gate matmul j on its W half only, so j 0-3 run while W's
            # second half is still in flight. Then 8 PE transposes to
            # u_T[hp, (k, b)] fp16 for the stream multiply.
            psum_uA = pe_pool.tile([B, 512], F32, tag="psum_e")
            psum_uB = pe_pool.tile([B, 512], F32, tag="psum_e")
            u32 = cp.tile([B, H], F32)
            for j in range(KH):
                lhs = hidT[:, 64 * j : 64 * (j + 1)]
                nc.tensor.matmul(
                    psum_uA[:], lhsT=lhs, rhs=wt[:, j * H : j * H + 512],
                    start=(j == 0), stop=(j == KH - 1),
                )
                nc.tensor.matmul(
                    psum_uB[:], lhsT=lhs, rhs=wt[:, j * H + 512 : (j + 1) * H],
                    start=(j == 0), stop=(j == KH - 1),
                )
            nc.scalar.copy(u32[:, 0:512], psum_uA[:])
            # uB eviction on DVE: first DVE instruction, resolves before the
            # first stream multiply's deps, so no head-of-line blocking
            nc.vector.tensor_copy(u32[:, 512:1024], psum_uB[:])
            u16T = cp.tile([128, 512], F16)
            for k in range(KH):
                psum_t = ppu.tile([128, B], F32, tag=f"ut{k % 2}")
                nc.tensor.transpose(
                    psum_t[:], u32[:, 128 * k : 128 * (k + 1)], idn[0:B, 0:B]
                )
                nc.scalar.copy(u16T[:, 64 * k : 64 * (k + 1)], psum_t[:])
            u3 = u16T[:].rearrange("p (k b) -> p k b", k=KH)

            # ---- c[b] = hidden[b] . bias, then broadcast over partitions;
            # emitted lazily after the first supergroups so it stays off the
            # PE/Scalar critical path (softmax only needs it much later) ----
            crep = cp.tile([128, B], F32)

            def emit_c_chain():
                psum_c = ppu.tile([1, B], F32, tag="ut0")
                for j in range(KH):
                    nc.tensor.matmul(
                        psum_c[:],
                        lhsT=biasT[:, j : j + 1],
                        rhs=hidT[:, 64 * j : 64 * (j + 1)],
                        start=(j == 0),
                        stop=(j == KH - 1),
                    )
                c_sb = cp.tile([1, B], F16)
                nc.scalar.copy(c_sb[:], psum_c[:])
                psum_crep = ppu.tile([128, B], F32, tag="ut1")
                nc.tensor.matmul(
                    psum_crep[:], lhsT=ones_row, rhs=c_sb[:], start=True, stop=True
                )
                nc.scalar.copy(crep[:], psum_crep[:])

            # ---- stream state ----
            e_sb = cp.tile([1, LS * B], F32)     # energies, l-major on part 0
            out_sb = cp.tile([B, LS], F32)

            # energies-transposed workspace tiles, one per softmax half;
            # filled by small per-supergroup SBUF->SBUF redistribute DMAs as
            # soon as each supergroup's evictions land, so the softmax only
            # waits on the last tiny one
            eth0 = cp.tile([LS // 2, B], F32)
            eth1 = cp.tile([LS // 2, B], F32)

            def redistribute(l0, ls):
                et_h = eth0 if l0 < LS // 2 else eth1
                lr = l0 % (LS // 2)
                nc.scalar.dma_start(
                    et_h[lr : lr + ls, :],
                    e_sb[0:1, l0 * B : (l0 + ls) * B].rearrange(
                        "p (l b) -> p l b", b=B
                    ),
                )

            def softmax_rows(hf, r0, r1, psum_o):
                # softmax along the free axis for l-rows [64hf+r0, 64hf+r1)
                # of half hf's [l-part, b-cols] workspace, into attn{hf}
                et_h = eth0 if hf == 0 else eth1
                nr = r1 - r0
                nc.vector.tensor_tensor(
                    out=et_h[r0:r1, :], in0=et_h[r0:r1, :], in1=crep[r0:r1, :],
                    op=ADD,
                )
                negm = cp.tile([LS // 2, 1], F32, tag=f"negm{hf}")
                nc.vector.tensor_reduce(
                    out=negm[r0:r1, :],
                    in_=et_h[r0:r1, :],
                    axis=mybir.AxisListType.X,
                    op=mybir.AluOpType.max,
                    negate=True,
                )
                pexp = cp.tile([LS // 2, B], F32, tag=f"pexp{hf}")
                ssum = cp.tile([LS // 2, 1], F32, tag=f"ssum{hf}")
                nc.scalar.activation(
                    pexp[r0:r1, :],
                    et_h[r0:r1, :],
                    mybir.ActivationFunctionType.Exp,
                    bias=negm[r0:r1, 0:1],
                    scale=1.0,
                    accum_out=ssum[r0:r1, :],
                )
                rs = cp.tile([LS // 2, 1], F32, tag=f"rs{hf}")
                nc.vector.reciprocal(rs[r0:r1, :], ssum[r0:r1, :])
                attn = cp.tile([LS // 2, B], F32, tag=f"attn{hf}")
                nc.vector.tensor_scalar_mul(
                    attn[r0:r1, :], pexp[r0:r1, :], rs[r0:r1, 0:1]
                )
                nc.tensor.transpose(
                    psum_o[:, r0:r1], attn[r0:r1, :], idn[r0:r1, r0:r1]
                )

            def softmax_out(hf, psum_o):
                nc.vector.tensor_copy(out_sb[:, 64 * hf : 64 * hf + 64], psum_o[:])
                # half 0 must stay OFF the sync ring: its wait would
                # head-of-line-block the remaining enc DMA issues behind it
                # in the ring FIFO; half 1 is emitted after all enc issues
                eng = nc.scalar if hf == 0 else nc.sync
                eng.dma_start(
                    out_p.ap()[:, 64 * hf : 64 * hf + 64],
                    out_sb[:, 64 * hf : 64 * hf + 64],
                )

            def softmax_half(hf):
                psum_o = ppu.tile([B, LS // 2], F32, tag="ut0")
                softmax_rows(hf, 0, LS // 2, psum_o)
                softmax_out(hf, psum_o)

            # ---- main stream ----
            l0 = 0
            off = 0
            for s, ls in enumerate(SG_L):
                cols = KH * ls * B           # tile cols (k, l, b)
                gcols = ls * B               # cols per h-chunk
                et = sp.tile([128, KH * 16 * B], F16, tag="et")
                nc.sync.dma_start(
                    et[:, 0:cols], enc_p.ap()[:, off : off + cols]
                )
                # per-h-chunk DVE multiply (fp16 2x, in place) interleaved
                # with the PE ones-reduce MMs of that chunk, so the PE gets
                # work every ~0.6us and HAM never re-throttles; the [1, 512]
                # PSUM groups stay open across the 8 chunks (start/stop)
                ngrp = (gcols + 511) // 512
                psum_es = []
                for g in range(ngrp):
                    psum_eg = pe_pool.tile([1, 512], F32, tag="psum_e")
                    psum_es.append(psum_eg)
                # chunk-pair DVE multiply: big enough to amortize the DVE
                # per-op overhead, small enough that the PE gets MM work
                # every ~1.2us (keeps HAM at K=8/8); tiny tail supergroups
                # use a single multiply to minimize per-op overhead
                # sg0: kstep=2 so the first multiply only needs u_T chunks
                # 0-1 (shortens the startup dependency chain)
                kstep = KH if ls <= 4 else (2 if s < 1 else 4)
                veng = nc.vector
                for k in range(0, KH, kstep):
                    ek = et[:, k * gcols : (k + kstep) * gcols].rearrange(
                        "p (ks l b) -> p ks l b", ks=kstep, b=B
                    )
                    veng.tensor_tensor(
                        out=ek,
                        in0=ek,
                        in1=u3[:, k : k + kstep, None, :].broadcast_to(
                            (128, kstep, ls, B)
                        ),
                        op=MULT,
                    )
                    for k2 in range(k, k + kstep):
                        for g in range(ngrp):
                            gn = min(512, gcols - 512 * g)
                            nc.tensor.matmul(
                                psum_es[g][:, 0:gn],
                                lhsT=ones_col,
                                rhs=et[:, k2 * gcols + 512 * g : k2 * gcols + 512 * g + gn],
                                start=(k2 == 0),
                                stop=(k2 == KH - 1),
                            )
                for g in range(ngrp):
                    gn = min(512, gcols - 512 * g)
                    nc.scalar.copy(
                        e_sb[0:1, l0 * B + 512 * g : l0 * B + 512 * g + gn],
                        psum_es[g][:, 0:gn],
                    )
                # tail supergroups share one batched redistribute: one issue
                # + one completion receipt on the exit-critical path
                if s < 9:
                    redistribute(l0, ls)
                elif s == len(SG_L) - 1:
                    redistribute(120, 8)
                l0 += ls
                off += cols
                if s == 1:
                    emit_c_chain()
                # half-0 softmax is emitted a few supergroups after its data
                # is complete: by then the half-0 redistribute receipts have
                # long landed, so its DVE ops can't head-of-line-block the
                # stream multiplies behind them in the DVE FIFO
                if s == 6:
                    softmax_half(0)
                # half-1 rows 0:32 (l 64:96) are complete after sg6's
                # redistribute; doing them here leaves only a [32, 64]
                # softmax slice on the exit-critical path
                if s == 8:
                    psum_o1 = ppu.tile([B, LS // 2], F32, tag="ut0")
                    softmax_rows(1, 0, 32, psum_o1)
            softmax_rows(1, 32, LS // 2, psum_o1)
            softmax_out(1, psum_o1)

    nc.compile()
    return nc


_IDENT = np.eye(128, dtype=np.float32)
_NC_CACHE = []


def _get_nc() -> bacc.Bacc:
    if not _NC_CACHE:
        _NC_CACHE.append(build_program())
    return _NC_CACHE[0]


def make_in_maps(hidden, encoder_outputs, W, b):
    # host side does only layout transforms + fp16 dtype compression; all
    # FLOPs stay on device (fp32 accumulation)
    hidden = np.asarray(hidden, dtype=np.float32)
    # hidT [128, (j, b)]: hidT[p, j*64+b] = hidden[b, 128j+p]
    hidT = hidden.T.reshape(KH, 128, B).transpose(1, 0, 2).reshape(128, 512)
    biasT = np.asarray(b, dtype=np.float32).reshape(KH, 128).T  # [128, 8]
    setup16 = np.zeros((128, SET_COLS), dtype=np.float16)
    setup16[:, 0:512] = hidT.astype(np.float16)
    setup16[:, 512:520] = biasT.astype(np.float16)
    setup16[:, 520:521] = 1.0
    setup16[:, 528:656] = 1.0
    # W o-chunk-major: w16[p, j*1024 + (128k+c)] = W[128j+p, 128k+c]
    w16 = (
        np.asarray(W, dtype=np.float32)
        .astype(np.float16)
        .reshape(KH, 128, H)            # [j, p, h]
        .transpose(1, 0, 2)             # [p, j, h]
        .reshape(128, KH * H)
    )
    w16 = np.ascontiguousarray(w16)
    setup32 = _IDENT

    enc16 = np.asarray(encoder_outputs, dtype=np.float32).astype(np.float16)
    in_maps = []
    for i in range(NCORES):
        # shard [l, b, h] -> [p, k, l, b] -> per-sg slabs [p, (k, l, b)]
        shard = enc16[i * LS : (i + 1) * LS].reshape(LS, B, KH, 128)
        shard = shard.transpose(3, 2, 0, 1)  # [p, k, l, b]
        slabs = []
        l0 = 0
        for ls in SG_L:
            slabs.append(
                np.ascontiguousarray(shard[:, :, l0 : l0 + ls, :]).reshape(128, -1)
            )
            l0 += ls
        encc = np.concatenate(slabs, axis=1)
        in_maps.append(
            {
                "setup16": setup16,
                "setup32": setup32,
                "w": w16,
                "enc": np.ascontiguousarray(encc),
            }
        )
    return in_maps


def kernel(hidden, encoder_outputs, W, b):
    nc = _get_nc()
    in_maps = make_in_maps(hidden, encoder_outputs, W, b)
    res = run_bass_kernel_spmd(nc, in_maps, core_ids=list(range(NCORES)))
    out = np.concatenate([res.results[i]["out"] for i in range(NCORES)], axis=1)
    return out[:, None, :].astype(np.float32)
